# revision 42
# baseline (speedup 1.0000x reference)
"""Trainium2 Bass kernel for CLIPAttention (B=32, S=512, E=768, H=12, D=64).

Strategy: data-parallel over batch across 8 NeuronCores (4 batches/core).
All matmul operands are fp16 (PSUM accumulates fp32); fp16 stationary
operands get fast-weight-load, which fp32/f32r weights do not.

hidden_states is pre-cast to fp16 AND pre-transposed to feature-major
[E, S] on the host (identical rounding to an on-chip cast, half the DMA
bytes, and no on-chip transposes at all). Weights are pre-cast to fp16.

Per batch:
  xT DMA'd straight in feature-major -> qT/kT feature-major + v token-major
  projections. Attention per head, with scores computed TRANSPOSED (k-major)
  so no transpose of the probabilities is ever needed:
    scoresT[k,q] = kh.T @ qh    (PE, triangular: only blocks with k <= q)
    pE = exp(scale * scoresT)   (ACT, written straight to SBUF as fp16)
    diagonal block masked by multiplying with an upper-triangular 0/1 tile
    den[q] = ones.T @ pE        (PE matmuls accumulating over k-tiles)
    po = v_h.T @ pE             (PE, triangular; unnormalized - the per-q
                                 normalization factors out of the k-sum)
    rden = approx-recip(den)    (DVE) -> broadcast to 128 partitions (GPSIMD)
    outT copy = po * rden       (DVE, fused into the PSUM->SBUF copy;
                                 partition-shifted writes put odd heads at
                                 partitions 64:127 directly)
  Final projection back to token-major; biases folded into PSUM->SBUF copies.

HW constraints learned the hard way (CoreSim does NOT model them):
  - engine reads of PSUM are only correct at partition base 0 and within a
    single 2KB bank (multi-bank APs and base-64 reads return garbage)
  - stationary (lhsT) free size must be a native tile width (<=32, 64, 128);
    65-68 wide stationaries load misaligned

The TRN2 PE drops to a half-speed p-state when its in-order queue stalls
and needs ~3us of continuous work to re-reach 2.4GHz, so emission order
software-pipelines attention two heads deep and interleaves independent
"filler" chains (v projection, previous batch's out projection) between
attention steps that depend on the scalar engine's exp results.
"""

import os
import time

import numpy as np
from contextlib import ExitStack

import concourse.bass as bass
import concourse.mybir as mybir
import concourse.tile as tile
from concourse import bacc
from concourse.bass_utils import run_bass_kernel_spmd
from concourse.masks import make_identity, make_upper_triangular

B, S, E, H, D = 32, 512, 768, 12, 64
NCORES = 8
NB = B // NCORES          # batches per core
P = 128
KT = E // P               # 6 feature tiles
QT = S // P               # 4 token tiles
SCALE = float(D) ** -0.5  # 0.125
F32 = mybir.dt.float32
F16 = mybir.dt.float16

AF = mybir.ActivationFunctionType
OP = mybir.AluOpType

NSPLIT = 384              # N-tile for the two token-major projections
HN = NSPLIT // D          # heads per N-chunk group = 6


def _build():
    nc = bacc.Bacc(trn_type="TRN2")

    # hidden_states arrives host-prepped as [NB, P, KT, S]: feature-major,
    # already in SBUF tile layout so each DMA descriptor is a contiguous
    # 6KB-per-partition run (128 fat descriptors, not 768 small ones)
    hs = nc.dram_tensor("hs", [NB, P, KT, S], F16, kind="ExternalInput")
    w_dr = {}
    # q/k weights host-prepped m-major [KT_m, P, KT_ko, P] so each per-m
    # block DMA is per-partition contiguous; v/o host-prepped [P, KT, E]
    for nm in ("q", "k"):
        w_dr[nm] = nc.dram_tensor(f"W{nm}", [KT, P, KT, P], F16,
                                  kind="ExternalInput")
    for nm in ("v", "o"):
        w_dr[nm] = nc.dram_tensor(f"W{nm}", [P, KT, E], F16,
                                  kind="ExternalInput")
    # biases arrive host-packed: [P, 6 (bq) + 6 (bk) + 768 (bv) + 768 (bo)];
    # bq/bk in per-partition form, bv/bo replicated to all partitions - one
    # contiguous DMA instead of 4-byte-element scatter patterns
    bias_dr = nc.dram_tensor("bias_pack", [P, 2 * KT + 2 * E], F32,
                             kind="ExternalInput")
    out = nc.dram_tensor("out", [NB, S, E], F32, kind="ExternalOutput")

    with ExitStack() as ctx:
        tc = ctx.enter_context(tile.TileContext(nc))

        singles = ctx.enter_context(tc.tile_pool(name="singles", bufs=1))
        xtpool = ctx.enter_context(tc.tile_pool(name="xtpool", bufs=2))
        qkvpool = ctx.enter_context(tc.tile_pool(name="qkvpool", bufs=2))
        pepool = ctx.enter_context(tc.tile_pool(name="pepool", bufs=4))
        rpool = ctx.enter_context(tc.tile_pool(name="rpool", bufs=4))
        otpool = ctx.enter_context(tc.tile_pool(name="otpool", bufs=2))
        opool = ctx.enter_context(tc.tile_pool(name="opool", bufs=2))

        ps_mm = ctx.enter_context(tc.tile_pool(name="ps_mm", bufs=2, space="PSUM"))
        ps_s = ctx.enter_context(tc.tile_pool(name="ps_s", bufs=3, space="PSUM"))
        ps_pv = ctx.enter_context(tc.tile_pool(name="ps_pv", bufs=2, space="PSUM"))
        ps_den = ctx.enter_context(tc.tile_pool(name="ps_den", bufs=1, space="PSUM"))

        # ---- constants ----
        # upper-triangular (incl diagonal) 0/1 mask: keeps q >= k entries of
        # a k-major diagonal block
        triu01 = singles.tile([P, P], F16, name="triu01")
        make_upper_triangular(nc, triu01, val=1.0, diag=True)
        ones16 = singles.tile([P, 1], F16, name="ones16")
        nc.vector.memset(ones16, 1.0)

        # batch-0 xT prefetched on the SCALAR engine's DGE ring, in parallel
        # with the weight DMAs on the sync ring
        xt_tiles = {}

        def load_xt(b, engine):
            t = xtpool.tile([P, KT, S], F16, name=f"xt_{b}", tag="xt")
            engine.dma_start(out=t, in_=hs[b])
            xt_tiles[b] = t

        # batch 0's xT rides the scalar engine's DGE ring, in parallel with
        # the weight burst on the sync ring (the scalar queue is idle at
        # startup); steady-state DMAs stay on the sync ring so their
        # dependency waits never block the exp stream
        load_xt(0, nc.scalar)

        # weights pre-cast to fp16 on the host; q/k are needed first and are
        # loaded in per-m column blocks, interleaved, so stage B's chain m
        # can start as soon as blocks q_m/k_m land. q/k SBUF layout is
        # m-major [P, KT_m, KT_ko, P]: lhsT of chain (m, kk) = [:, m, kk, :]
        w_sb = {}
        for nm in ("q", "k"):
            w_sb[nm] = singles.tile([P, KT, KT, P], F16, name=f"w_{nm}")
        for nm in ("v", "o"):
            w_sb[nm] = singles.tile([P, KT, E], F16, name=f"w_{nm}")

        for m in range(KT):
            for nm in ("q", "k"):
                nc.sync.dma_start(out=w_sb[nm][:, m, :, :], in_=w_dr[nm][m])
        bias_sb = singles.tile([P, 2 * KT + 2 * E], F32, name="bias_sb")
        nc.sync.dma_start(out=bias_sb, in_=bias_dr[:, :])
        for nm in ("v", "o"):
            nc.sync.dma_start(out=w_sb[nm], in_=w_dr[nm][:, :, :])
        bias_pp = {"q": bias_sb[:, 0:KT], "k": bias_sb[:, KT:2 * KT]}
        bias_bc = {
            "v": bias_sb[:, 2 * KT:2 * KT + E],
            "o": bias_sb[:, 2 * KT + E:2 * KT + 2 * E],
        }

        prev_E = []  # deferred out-projection chains of the previous batch
        pulled_qkv = None  # next batch's qT/kT when its m=0 chains are pulled

        for b in range(NB):
            xt = xt_tiles.pop(b)
            if b + 1 < NB:
                load_xt(b + 1, nc.sync)

            # ---- stage B: qT, kT feature-major [768, 512] ----
            # chain (nm, m); force_dve keeps pulled-forward chains off the
            # scalar queue (which carries the exp stream during attention)
            def make_b_chain(nm, m, qkv_t, xt_t, b=b, force_dve=False):
                def emit():
                    dst = qkv_t[nm]
                    ps = ps_mm.tile([P, S], F32, name=f"ps{nm}_{b}_{m}", tag="mm")
                    for kk in range(KT):
                        nc.tensor.matmul(
                            ps,
                            lhsT=w_sb[nm][:, m, kk, :],
                            rhs=xt_t[:, kk, :],
                            start=(kk == 0),
                            stop=(kk == KT - 1),
                        )
                    if m % 2 == 0 and not force_dve:
                        nc.scalar.activation(
                            out=dst[:, m, :],
                            in_=ps,
                            func=AF.Identity,
                            bias=bias_pp[nm][:, m:m + 1],
                            scale=1.0,
                        )
                    else:
                        nc.vector.tensor_scalar_add(
                            out=dst[:, m, :],
                            in0=ps,
                            scalar1=bias_pp[nm][:, m:m + 1],
                        )
                return emit

            if pulled_qkv is None:
                qkv = {}
                for nm in ("q", "k"):
                    qkv[nm] = qkvpool.tile(
                        [P, KT, S], F16, name=f"{nm}T_{b}", tag=f"{nm}T"
                    )
                m_start = 0
            else:
                qkv, m_start = pulled_qkv  # pulled chains ran in previous D
            for m in range(m_start, KT):
                for nm in ("q", "k"):
                    make_b_chain(nm, m, qkv, xt)()

            # pre-allocate next batch's qT/kT and build its leading chains
            # to fill this batch's attention steps; batch 0 has no prev_E
            # filler, so it pulls three m-blocks instead of one
            if b + 1 < NB:
                qkv_next = {}
                for nm in ("q", "k"):
                    qkv_next[nm] = qkvpool.tile(
                        [P, KT, S], F16, name=f"{nm}T_{b + 1}", tag=f"{nm}T"
                    )
                xt_next = xt_tiles[b + 1]
                pull_ms = (0, 1, 2, 3, 4, 5) if b == 0 else (0,)
                b_pulls = [
                    make_b_chain(nm, m, qkv_next, xt_next, b=b + 1,
                                 force_dve=True)
                    for m in pull_ms for nm in ("q", "k")
                ]
                pulled_qkv = (qkv_next, len(pull_ms))
            else:
                pulled_qkv = None
                b_pulls = []

            # ---- stage C (deferred chains): v token-major [512, 768] ----
            v_t = qkvpool.tile([P, QT, E], F16, name=f"v_{b}", tag="v")

            def make_c_chain(i, n, b=b, xt=xt, v_t=v_t):
                def emit():
                    ps = ps_mm.tile([P, S], F32, name=f"psv_{b}_{i}_{n}", tag="mm")
                    for kk in range(KT):
                        nc.tensor.matmul(
                            ps[:, :NSPLIT],
                            lhsT=xt[:, kk, i * P:(i + 1) * P],
                            rhs=w_sb["v"][:, kk, n * NSPLIT:(n + 1) * NSPLIT],
                            start=(kk == 0),
                            stop=(kk == KT - 1),
                        )
                    nc.vector.tensor_tensor(
                        out=v_t[:, i, n * NSPLIT:(n + 1) * NSPLIT],
                        in0=ps[:, :NSPLIT],
                        in1=bias_bc["v"][:, n * NSPLIT:(n + 1) * NSPLIT],
                        op=OP.add,
                    )
                return emit

            c_chains = [make_c_chain(i, n) for n in range(2) for i in range(QT)]

            # ---- stage D: attention heads (k-major probs, no transposes) ----
            # software-pipelined two heads deep: head h's den/PV are emitted
            # after head h+2's scores, so the PE has independent matmul work
            # while the scalar engine's exp chain catches up
            outT = otpool.tile([P, KT, S], F16, name=f"outT_{b}", tag="outT")
            pE_live = {}

            def emit_scores(h, b=b, qkv=qkv, pE_live=pE_live):
                g, rr = h // 2, h % 2
                qh = qkv["q"][rr * D:(rr + 1) * D, g, :]
                kh = qkv["k"][rr * D:(rr + 1) * D, g, :]
                pE = pepool.tile([P, QT, S], F16, name=f"pE_{b}_{h}", tag="pE")
                pE_live[h] = pE
                for j in range(QT):
                    q0 = j * P
                    n_mm = S - q0
                    ps = ps_s.tile([P, S], F32, name=f"pss_{b}_{h}_{j}", tag="s")
                    nc.tensor.matmul(
                        ps[:, :n_mm],
                        lhsT=kh[:, j * P:(j + 1) * P],
                        rhs=qh[:, q0:],
                        start=True,
                        stop=True,
                    )
                    nc.scalar.activation(
                        out=pE[:, j, q0:],
                        in_=ps[:, :n_mm],
                        func=AF.Exp,
                        scale=SCALE,
                    )
                    # causal mask on the diagonal block: keep q >= k
                    nc.vector.tensor_tensor(
                        out=pE[:, j, q0:q0 + P],
                        in0=pE[:, j, q0:q0 + P],
                        in1=triu01,
                        op=OP.mult,
                    )

            def emit_pv(h, b=b, v_t=v_t, outT=outT, pE_live=pE_live):
                g, rr = h // 2, h % 2
                pE = pE_live.pop(h)
                # denominator: ones.T @ pE accumulated over k-tiles
                den = ps_den.tile([1, S], F32, name=f"den_{b}_{h}", tag="den")
                for j in range(QT):
                    nc.tensor.matmul(
                        den[:, j * P:],
                        lhsT=ones16,
                        rhs=pE[:, j, j * P:],
                        start=(j == 0),
                        stop=(j == QT - 1),
                        skip_group_check=True,
                    )
                # PV, unnormalized, triangular over valid k-ranges
                po = ps_pv.tile([D, S], F32, name=f"po_{b}_{h}", tag="pv")
                for j in range(QT):
                    nc.tensor.matmul(
                        po[:, j * P:],
                        lhsT=v_t[:, j, h * D:(h + 1) * D],
                        rhs=pE[:, j, j * P:],
                        start=(j == 0),
                        stop=(j == QT - 1),
                        skip_group_check=True,
                    )
                rden = rpool.tile([1, S], F32, name=f"rden_{b}_{h}", tag="rden")
                nc.vector.reciprocal_approx_fast(rden, den)
                rb = rpool.tile([P, S], F32, name=f"rb_{b}_{h}", tag="rb")
                nc.gpsimd.partition_broadcast(rb, rden)
                # normalization fused into the PSUM->SBUF copy; the write is
                # partition-shifted for odd heads (engines support src/dst
                # partition bases differing)
                nc.vector.tensor_tensor(
                    out=outT[rr * D:(rr + 1) * D, g, :],
                    in0=po,
                    in1=rb[0:D, :],
                    op=OP.mult,
                )

            # filler schedule: C chains early (PV of head h needs v chunk
            # n = h // 6), previous batch's out-projection after
            # C chains early (PV of head h needs v chunk n = h // 6);
            # previous batch's out-projection late, reaching into the two
            # drain steps so the PE stays fed while the last heads' exps
            # trail
            filler = {
                0: c_chains[0:2],          # n=0: heads 0-5
                1: c_chains[2:4],
                2: c_chains[4:6],          # n=1: heads 6-11
                3: c_chains[6:8],
            }
            for idx, ch in enumerate(b_pulls):
                filler.setdefault(min(4 + idx, 13), []).append(ch)
            for idx, ch in enumerate(prev_E):
                filler.setdefault(6 + idx, []).append(ch)
            for step in range(H + 2):
                if step < H:
                    emit_scores(step)
                for emit in filler.get(step, ()):
                    emit()
                if step >= 2:
                    emit_pv(step - 2)

            # ---- stage E (deferred to batch b+1's attention phase): final
            # projection, token-major out ----
            def make_e_chain(i, n, b=b, outT=outT):
                def emit():
                    o_t = opool.tile([P, NSPLIT], F32, name=f"o_{b}_{i}_{n}", tag="o")
                    ps = ps_mm.tile([P, S], F32, name=f"pso_{b}_{i}_{n}", tag="mm")
                    for kk in range(KT):
                        nc.tensor.matmul(
                            ps[:, :NSPLIT],
                            lhsT=outT[:, kk, i * P:(i + 1) * P],
                            rhs=w_sb["o"][:, kk, n * NSPLIT:(n + 1) * NSPLIT],
                            start=(kk == 0),
                            stop=(kk == KT - 1),
                        )
                    nc.vector.tensor_tensor(
                        out=o_t,
                        in0=ps[:, :NSPLIT],
                        in1=bias_bc["o"][:, n * NSPLIT:(n + 1) * NSPLIT],
                        op=OP.add,
                    )
                    nc.sync.dma_start(
                        out=out[b, i * P:(i + 1) * P, n * NSPLIT:(n + 1) * NSPLIT],
                        in_=o_t,
                    )
                return emit

            prev_E = [make_e_chain(i, n) for i in range(QT) for n in range(2)]

        # tail: the last batch's out-projection
        for emit in prev_E:
            emit()

    nc.compile()
    return nc


_NC_CACHE = None


def _get_nc():
    global _NC_CACHE
    if _NC_CACHE is None:
        _NC_CACHE = _build()
    return _NC_CACHE


def prep_inputs(inputs):
    # hidden_states pre-cast to fp16 and pre-arranged to the SBUF tile
    # layout [B, P, KT, S] on the host: identical rounding to the on-chip
    # cast, half the DMA bytes, no on-chip transposes, and per-partition
    # contiguous DMA descriptors
    hs = np.asarray(inputs["hidden_states"], dtype=np.float32).astype(np.float16)
    hsT = hs.transpose(0, 2, 1).reshape(B, KT, P, S)   # [B, ko, p, S]
    hsT = np.ascontiguousarray(hsT.transpose(0, 2, 1, 3))  # [B, P, KT, S]
    wb = {}
    for nm in ("q", "k"):
        w = np.asarray(inputs[f"W{nm}"], dtype=np.float32).astype(np.float16)
        # [m, p, ko, c]: per-m column block in SBUF tile layout
        wb[f"W{nm}"] = np.ascontiguousarray(
            w.reshape(KT, P, KT, P).transpose(2, 1, 0, 3)
        )
    for nm in ("v", "o"):
        w = np.asarray(inputs[f"W{nm}"], dtype=np.float32).astype(np.float16)
        wb[f"W{nm}"] = np.ascontiguousarray(
            w.reshape(KT, P, E).transpose(1, 0, 2)     # [p, ko, e]
        )
    bq = np.asarray(inputs["bq"], np.float32).reshape(KT, P).T        # [P, KT]
    bk = np.asarray(inputs["bk"], np.float32).reshape(KT, P).T
    bv = np.broadcast_to(np.asarray(inputs["bv"], np.float32), (P, E))
    bo = np.broadcast_to(np.asarray(inputs["bo"], np.float32), (P, E))
    wb["bias_pack"] = np.ascontiguousarray(
        np.concatenate([bq, bk, bv, bo], axis=1)
    )
    return hsT, wb


def run(inputs, trace=False):
    if trace:
        os.environ.pop("BASS_NEVER_TRACE", None)
    else:
        # keep the spmd runner off the NTFF trace path (the profiling hook
        # module is not always present)
        os.environ["BASS_NEVER_TRACE"] = "1"
    hsT, wb = prep_inputs(inputs)

    nc = _get_nc()
    in_maps = []
    for c in range(NCORES):
        m = {"hs": hsT[c * NB:(c + 1) * NB]}
        m.update(wb)
        in_maps.append(m)
    res = run_bass_kernel_spmd(
        nc, in_maps, core_ids=list(range(NCORES)), trace=trace
    )
    outp = np.concatenate([r_["out"] for r_ in res.results], axis=0)
    return outp, res


def kernel(**inputs) -> np.ndarray:
    # retry once on transient accelerator errors (rare NRT exec glitches)
    last = None
    for attempt in range(2):
        try:
            outp, _ = run(inputs, trace=False)
            return outp
        except Exception as e:  # noqa: BLE001
            last = e
            time.sleep(10)
    raise last
